# revision 1
# baseline (speedup 1.0000x reference)
"""Self-contained ChildSum TreeLSTM kernel for 8 Trainium2 NeuronCores.

Strategy: subtree-partitioned data parallelism. Bulk nodes (small subtrees,
low levels) are packed into 8 per-core forests (all child->parent edges
core-local); the top "tail" nodes are replicated on every core. Each core
computes partial child-sum aggregates for tail nodes from its own bulk
children, one AllReduce combines them, then every core runs the tail levels
identically. Level-synchronous tiles of 128 nodes; child states gathered by
indirect DMA from the HBM [h | h@W_fh.T | c] row store (which doubles as the
output tensor; the host slices out h); all matmuls in float32r (TF32-like)
with f32 PSUM accumulation.

kernel(**inputs) takes the full unsharded inputs and returns the full
[N, 150] float32 output h for every node.
"""
import numpy as np

"""Host-side schedule construction for the ChildSum TreeLSTM Trainium kernel.

Partitioning strategy (SPMD across 8 cores):
  - "bulk" nodes (low tree levels, small subtrees) are partitioned into
    per-core forests of whole subtrees -> all child gathers are core-local.
  - "tail" nodes R = {level >= L_CUT or subtree_size > SIZE_CAP} are
    replicated on every core.  Each core computes partial child-sum
    aggregates (h_sum, sum f*c) for tail nodes from its own bulk children
    (the "pseudo level"), an AllReduce sums them, then every core runs the
    tail levels identically.

Per-core node slots (uniform layout across cores):
  [bulk levels 0..L_CUT-1, each padded to tiles of 128] ++
  [tail levels, padded] ++ [zero row]

All schedule arrays are baked either into the Bass program (constants) or
into per-core int32 input tensors (gather indices).
"""

import numpy as np

P = 128


class Schedule:
    pass


def _tiles(n):
    return max(1, (n + P - 1) // P)


def build_schedule(parents, n_cores=8, size_cap=None, level_cut=None):
    parents = np.asarray(parents, dtype=np.int64)
    N = parents.shape[0]

    # --- children CSR, levels, subtree sizes (parents[i] > i topological)
    level = np.zeros(N, np.int32)
    size = np.ones(N, np.int64)
    deg = np.zeros(N + 1, np.int64)
    for i in range(N - 1):
        p = parents[i]
        lp = level[i] + 1
        if lp > level[p]:
            level[p] = lp
        size[p] += size[i]
        deg[p] += 1
    deg[parents[N - 1]] += 0  # root's parent is dummy N
    height = int(level.max())

    # children lists sorted by child index (CSR over parent)
    order_by_parent = np.argsort(parents[: N - 1], kind="stable")
    csr_off = np.zeros(N + 2, np.int64)
    np.add.at(csr_off, parents[: N - 1] + 1, 1)
    csr_off = np.cumsum(csr_off)
    csr_children = np.empty(N - 1, np.int64)
    pos = csr_off[:-1].copy()
    # stable fill: iterate sorted-by-parent order; order_by_parent gives
    # children grouped by parent, ascending child index within parent.
    csr_children[:] = order_by_parent

    def children_of(j):
        return csr_children[csr_off[j]: csr_off[j + 1]]

    # --- choose cut parameters
    if level_cut is None:
        # smallest level where global count <= n_cores * 40
        counts = np.bincount(level, minlength=height + 1)
        level_cut = height + 1
        for l in range(height + 1):
            if counts[l] <= n_cores * 40:
                level_cut = l
                break
    if size_cap is None:
        size_cap = max(256, int(N // (n_cores * 1.35)))

    is_tail = (level >= level_cut) | (size > size_cap)
    # tail must be upward-closed: parent of a tail node is tail
    # (parent has higher level -> level>=level_cut holds if child tail by
    #  level; if child tail by size, parent size > child size > cap ✓)
    tail_nodes = np.nonzero(is_tail)[0]
    bulk_mask = ~is_tail

    # --- bulk subtree roots: bulk nodes whose parent is tail (or dummy)
    sub_roots = [i for i in np.nonzero(bulk_mask)[0]
                 if parents[i] == N or is_tail[parents[i]]]
    # collect subtree node sets via descending pass: nodes' root pointer
    root_of = np.full(N, -1, np.int64)
    for r in sub_roots:
        root_of[r] = r
    # descending index order: parent has higher index, so process descending
    # to propagate root pointers down.  parent[i] > i so child i gets its
    # parent's root AFTER parent processed -> iterate i descending.
    for i in range(N - 2, -1, -1):
        if bulk_mask[i] and root_of[i] == -1:
            root_of[i] = root_of[parents[i]]

    # --- LPT bin packing of subtrees into n_cores bins
    sub_roots_arr = np.array(sub_roots, np.int64)
    sub_sizes = size[sub_roots_arr]
    order = np.argsort(-sub_sizes, kind="stable")
    bin_tot = np.zeros(n_cores, np.int64)
    bin_of_root = {}
    for k in order:
        b = int(np.argmin(bin_tot))
        bin_of_root[int(sub_roots_arr[k])] = b
        bin_tot[b] += sub_sizes[k]
    core_of = np.full(N, -1, np.int8)
    bm = np.nonzero(bulk_mask)[0]
    core_of[bm] = [bin_of_root[int(root_of[i])] for i in bm]

    # --- per-core per-level node lists, degree-sorted (desc), id asc
    bulk_levels = sorted(set(level[bulk_mask].tolist()))
    # bulk levels are 0..level_cut-1 potentially with gaps; keep actual set
    core_level_nodes = [[[] for _ in range(level_cut)] for _ in range(n_cores)]
    for i in bm:
        core_level_nodes[core_of[i]][level[i]].append(int(i))
    for c in range(n_cores):
        for l in range(level_cut):
            core_level_nodes[c][l].sort(key=lambda i: (-deg[i], i))

    # --- tail levels (global, replicated), degree-sorted by tail-internal deg
    tdeg = np.zeros(N, np.int64)   # number of tail children of a tail node
    for i in tail_nodes:
        p = parents[i]
        if p < N:
            tdeg[p] += 1
    tail_levels = sorted(set(level[tail_nodes].tolist()))
    tail_level_nodes = {}
    for l in tail_levels:
        ns = [int(i) for i in tail_nodes if level[i] == l]
        ns.sort(key=lambda i: (-tdeg[i], i))
        tail_level_nodes[l] = ns

    # --- slot layout
    # bulk: per level padded to max core count -> tiles
    s = Schedule()
    s.N, s.n_cores, s.height = N, n_cores, height
    s.level_cut, s.size_cap = level_cut, size_cap
    s.parents, s.level, s.deg, s.core_of = parents, level, deg, core_of

    slot_of = [np.full(N + 1, -1, np.int64) for _ in range(n_cores)]
    bulk_level_info = []   # (level, base_slot, n_tiles, counts_per_core)
    cur = 0
    for l in range(level_cut):
        mx = max(len(core_level_nodes[c][l]) for c in range(n_cores))
        if mx == 0:
            continue
        nt = _tiles(mx)
        for c in range(n_cores):
            for r, i in enumerate(core_level_nodes[c][l]):
                slot_of[c][i] = cur + r
        bulk_level_info.append((l, cur, nt,
                               [len(core_level_nodes[c][l]) for c in range(n_cores)]))
        cur += nt * P
    s.S_bulk = cur

    # tail slots (same on all cores)
    tail_level_info = []
    for l in tail_levels:
        ns = tail_level_nodes[l]
        nt = _tiles(len(ns))
        for r, i in enumerate(ns):
            for c in range(n_cores):
                slot_of[c][i] = cur + r
        tail_level_info.append((l, cur, nt, len(ns)))
        cur += nt * P
    s.S_tail = cur - s.S_bulk
    s.zero_slot = cur
    s.S_total = cur + P   # one extra padded tile row-block; row `cur` is the zero row

    s.slot_of = slot_of
    s.bulk_level_info = bulk_level_info
    s.tail_level_info = tail_level_info
    s.core_level_nodes = core_level_nodes
    s.tail_level_nodes = tail_level_nodes

    # --- R ordering for the partial/allreduce buffer: level-major (same as
    # tail slot order), position of tail node in concatenated tail lists
    rpos = {}
    rp = 0
    tail_concat = []
    for l in tail_levels:
        for i in tail_level_nodes[l]:
            rpos[i] = rp
            tail_concat.append(i)
            rp += 1
    s.R_count = rp
    s.R_rows = _tiles(rp) * P + P    # + junk tile for dummy scatter targets
    s.rpos = rpos
    s.tail_concat = tail_concat

    # --- pseudo level: per-core boundary parents (tail nodes w/ bulk children
    # in this core), with per-core boundary-degree-sorted order
    bnd_children = [dict() for _ in range(n_cores)]   # core -> {tail j: [bulk children]}
    for i in bm:
        p = int(parents[i])
        if p < N and is_tail[p]:
            bnd_children[core_of[i]].setdefault(p, []).append(int(i))
    pseudo_nodes = []   # per core: sorted list of (j, children list)
    for c in range(n_cores):
        items = sorted(bnd_children[c].items(),
                       key=lambda kv: (-len(kv[1]), kv[0]))
        for j, ch in items:
            ch.sort()
        pseudo_nodes.append(items)
    s.pseudo_nodes = pseudo_nodes
    s.pseudo_count = max((len(pn) for pn in pseudo_nodes), default=0)
    s.pseudo_tiles = _tiles(s.pseudo_count) if s.pseudo_count else 0

    # --- tail-internal children (for tail level rounds)
    tail_children = {}   # tail j -> [tail children]
    for i in tail_nodes:
        p = int(parents[i])
        if p < N:
            tail_children.setdefault(p, []).append(int(i))
    for j in tail_children:
        tail_children[j].sort()
    s.tail_children = tail_children

    def bulk_children_sorted(c, j):
        return [i for i in children_of(j) if core_of[i] == c]

    s.children_of = children_of
    return s


def build_gather_plan(s):
    """Create, per core, the flat int32 index stream plus the per-(phase,
    level,tile) metadata of [n_rounds, prefix lengths], identical across
    cores structurally.

    Index stream blocks of P entries each, in emission order:
      for each bulk level, tile: for each round: P child-slot indices
      pseudo: for each tile: [P scatter-target rows] then per round P indices
      for each tail level, tile: per round P indices
    Returns (meta, idx_streams) where idx_streams is [n_cores][n_blocks*P]
    """
    n_cores, P_ = s.n_cores, P
    Z = s.zero_slot
    streams = [[] for _ in range(n_cores)]

    def emit_block(vals_per_core):
        for c in range(n_cores):
            v = vals_per_core[c]
            assert len(v) == P_
            streams[c].extend(v)

    meta = {"bulk": [], "pseudo": [], "tail": []}
    nblocks = 0

    # ---- bulk levels
    for (l, base, nt, counts) in s.bulk_level_info:
        tiles_meta = []
        for t in range(nt):
            r0, r1 = t * P_, (t + 1) * P_
            # per-core rows in this tile: node lists
            rows = []
            for c in range(n_cores):
                nodes = s.core_level_nodes[c][l][r0:r1]
                rows.append(nodes)
            if l == 0:
                maxdeg = 0
            else:
                maxdeg = max((int(s.deg[i]) for c in range(n_cores)
                              for i in rows[c]), default=0)
            rounds = []
            for k in range(maxdeg):
                # prefix length: rows with deg > k (max over cores)
                mk = 0
                for c in range(n_cores):
                    cnt = sum(1 for i in rows[c] if s.deg[i] > k)
                    mk = max(mk, cnt)
                if mk == 0:
                    break
                blocks = []
                for c in range(n_cores):
                    v = []
                    for r in range(P_):
                        if r < len(rows[c]) and s.deg[rows[c][r]] > k:
                            ch = s.children_of(rows[c][r])
                            # children all bulk&same core OR... (bulk node children are same-core bulk)
                            v.append(int(s.slot_of[c][ch[k]]))
                        else:
                            v.append(Z)
                    blocks.append(v)
                emit_block(blocks)
                rounds.append((mk, nblocks))
                nblocks += 1
            tiles_meta.append((t, rounds))
        meta["bulk"].append((l, base, nt, tiles_meta))

    # ---- pseudo level
    pseudo_meta = []
    for t in range(s.pseudo_tiles):
        r0, r1 = t * P_, (t + 1) * P_
        rows = [s.pseudo_nodes[c][r0:r1] for c in range(len(s.pseudo_nodes))]
        # scatter-target block: partial-buffer row for each tile row
        blocks = []
        for c in range(n_cores):
            v = []
            for r in range(P_):
                if r < len(rows[c]):
                    v.append(int(s.rpos[rows[c][r][0]]))
                else:
                    v.append(s.R_rows - P_ + r)   # junk tile rows
            blocks.append(v)
        emit_block(blocks)
        scatter_block = nblocks
        nblocks += 1
        maxdeg = max((len(ch) for c in range(n_cores) for (_, ch) in rows[c]),
                     default=0)
        rounds = []
        for k in range(maxdeg):
            mk = 0
            for c in range(n_cores):
                cnt = sum(1 for (_, ch) in rows[c] if len(ch) > k)
                mk = max(mk, cnt)
            if mk == 0:
                break
            blocks = []
            for c in range(n_cores):
                v = []
                for r in range(P_):
                    if r < len(rows[c]) and len(rows[c][r][1]) > k:
                        v.append(int(s.slot_of[c][rows[c][r][1][k]]))
                    else:
                        v.append(Z)
                blocks.append(v)
            emit_block(blocks)
            rounds.append((mk, nblocks))
            nblocks += 1
        pseudo_meta.append((t, scatter_block, rounds))
    meta["pseudo"] = pseudo_meta

    # ---- tail levels
    for (l, base, nt, count) in s.tail_level_info:
        tiles_meta = []
        nodes_all = s.tail_level_nodes[l]
        for t in range(nt):
            rows = nodes_all[t * P_:(t + 1) * P_]
            maxdeg = max((len(s.tail_children.get(i, [])) for i in rows),
                         default=0)
            rounds = []
            for k in range(maxdeg):
                mk = sum(1 for i in rows if len(s.tail_children.get(i, [])) > k)
                if mk == 0:
                    break
                blocks = []
                v = []
                for r in range(P_):
                    if r < len(rows) and len(s.tail_children.get(rows[r], [])) > k:
                        v.append(int(s.slot_of[0][s.tail_children[rows[r]][k]]))
                    else:
                        v.append(Z)
                for c in range(n_cores):
                    blocks.append(list(v))
                emit_block(blocks)
                rounds.append((mk, nblocks))
                nblocks += 1
            tiles_meta.append((t, rounds))
        meta["tail"].append((l, base, nt, tiles_meta))

    idx_streams = [np.array(st, np.int32).reshape(-1, 1) for st in streams]
    if nblocks == 0:
        idx_streams = [np.zeros((P_, 1), np.int32) for _ in range(n_cores)]
        nblocks = 1
    return meta, idx_streams, nblocks


def build_x_inputs(s, x, dt=np.float32):
    """Per-core transposed/augmented x: [301, S_x] where columns are
    [bulk slots | tail slots | pseudo slots].  Row 300 = 1.0 for real cols.
    Returns list of arrays + S_x and pseudo column base."""
    N, n_cores = s.N, s.n_cores
    IN = x.shape[1]
    S_x = s.S_bulk + s.S_tail + s.pseudo_tiles * P
    outs = []
    for c in range(n_cores):
        xa = np.zeros((IN + 1, S_x), np.float32)
        # bulk + tail via slot_of
        nodes = np.nonzero(s.slot_of[c][:N] >= 0)[0]
        slots = s.slot_of[c][nodes]
        xa[:IN, slots] = x[nodes].T
        xa[IN, slots] = 1.0
        # pseudo cols
        pb = s.S_bulk + s.S_tail
        for r, (j, _) in enumerate(s.pseudo_nodes[c]):
            xa[:IN, pb + r] = x[j]
            xa[IN, pb + r] = 1.0
        outs.append(xa.astype(dt))
    return outs, S_x, s.S_bulk + s.S_tail





from contextlib import ExitStack

import numpy as np

import concourse.bass as bass
import concourse.bacc as bacc
import concourse.mybir as mybir
import concourse.tile as tile
from concourse.masks import make_identity

P = 128
F32 = mybir.dt.float32
F32R = mybir.dt.float32r
BF16 = mybir.dt.bfloat16
I32 = mybir.dt.int32
SIG = mybir.ActivationFunctionType.Sigmoid
TANH = mybir.ActivationFunctionType.Tanh

DT = {"bf16": BF16, "f32r": F32R, "f32": F32}


def build_kernel(s, meta, nblocks, IN=300, M=150, n_cores=8,
                 mm_dt="f32r", hc_dt="f32r"):
    MM, HC = DT[mm_dt], DT[hc_dt]
    KA = IN + 1
    M3, M4 = 3 * M, 4 * M
    ROW = 3 * M
    S_slots = s.S_bulk + s.S_tail
    S_x = S_slots + s.pseudo_tiles * P
    kchunks = [(i, min(P, KA - i)) for i in range(0, KA, P)]
    mchunks = [(i, min(P, M - i)) for i in range(0, M, P)]

    max_tiles = max([len(tm) for (_, _, _, tm) in meta["bulk"] + meta["tail"]]
                    + [len(meta["pseudo"])])
    max_rounds = max([sum(len(r) for (_, r) in tm)
                      for (_, _, _, tm) in meta["bulk"] + meta["tail"]]
                     + [sum(len(r) for (_, _, r) in meta["pseudo"])])

    nc = bacc.Bacc("TRN2", target_bir_lowering=False, debug=False,
                   num_devices=n_cores, num_swdge_queues=4)

    x_t = nc.dram_tensor("x_t", [KA, S_x], MM, kind="ExternalInput")
    idx_d = nc.dram_tensor("idx", [P, nblocks], I32, kind="ExternalInput")
    w_all_t = nc.dram_tensor("w_all_t", [KA, M4], MM, kind="ExternalInput")
    w_iouh_t = nc.dram_tensor("w_iouh_t", [M, M3], HC, kind="ExternalInput")
    w_fh_t = nc.dram_tensor("w_fh_t", [M, M], HC, kind="ExternalInput")
    hc_d = nc.dram_tensor("hc", [S_slots + P, ROW], HC, kind="ExternalOutput")
    partial_d = nc.dram_tensor("partial", [s.R_rows, 2 * M], F32, kind="Internal")
    reduced_d = nc.dram_tensor("reduced", [s.R_rows, 2 * M], F32, kind="Internal",
                               addr_space="Shared")

    with tile.TileContext(nc) as tc, ExitStack() as ctx:
        const = ctx.enter_context(tc.tile_pool(name="const", bufs=1))
        sbL = ctx.enter_context(tc.tile_pool(name="sbL", bufs=max_tiles + 3))
        sbG = ctx.enter_context(tc.tile_pool(name="sbG", bufs=max_rounds + 3))
        sbC = ctx.enter_context(tc.tile_pool(name="sbC", bufs=6))
        sbO = ctx.enter_context(tc.tile_pool(name="sbO", bufs=4))
        psA = ctx.enter_context(tc.tile_pool(name="psA", bufs=2, space="PSUM"))
        psB = ctx.enter_context(tc.tile_pool(name="psB", bufs=2, space="PSUM"))
        psC = ctx.enter_context(tc.tile_pool(name="psC", bufs=2, space="PSUM"))
        psT = ctx.enter_context(tc.tile_pool(name="psT", bufs=2, space="PSUM"))

        # ---------- constants ----------
        ident_f = const.tile([P, P], F32)
        make_identity(nc, ident_f[:])
        ident = const.tile([P, P], HC)
        nc.vector.tensor_copy(out=ident[:], in_=ident_f[:])

        w_all_sb = []
        for (k0, kn) in kchunks:
            t = const.tile([P, M4], MM, tag=f"wall{k0}")
            nc.sync.dma_start(out=t[:kn, :], in_=w_all_t.ap()[k0:k0 + kn, :])
            w_all_sb.append(t)
        w_iouh_sb, w_fh_sb = [], []
        for (m0, mn) in mchunks:
            t = const.tile([P, M3], HC, tag=f"wiouh{m0}")
            nc.sync.dma_start(out=t[:mn, :], in_=w_iouh_t.ap()[m0:m0 + mn, :])
            w_iouh_sb.append(t)
            t2 = const.tile([P, M], HC, tag=f"wfh{m0}")
            nc.sync.dma_start(out=t2[:mn, :], in_=w_fh_t.ap()[m0:m0 + mn, :])
            w_fh_sb.append(t2)

        idx_sb = const.tile([P, nblocks], I32)
        nc.sync.dma_start(out=idx_sb[:], in_=idx_d.ap())

        zero_f32 = const.tile([P, ROW], F32)
        nc.gpsimd.memset(zero_f32[:], 0.0)
        zero_hc = const.tile([P, ROW], HC)
        nc.vector.tensor_copy(out=zero_hc[:], in_=zero_f32[:])
        nc.sync.dma_start(out=hc_d.ap()[S_slots:S_slots + P, :], in_=zero_hc[:])
        for r0 in range(0, s.R_rows, P):
            nc.sync.dma_start(out=partial_d.ap()[r0:r0 + P, :],
                              in_=zero_f32[:, :2 * M])

        # ---------- helpers ----------
        def xT_load(col0):
            xt = sbL.tile([P, len(kchunks) * P], MM, tag="xT")
            # chunks 0..1 fused in one 3D-AP DMA; chunk 2 separate
            nc.sync.dma_start(
                out=xt[:, 0:2 * P].rearrange("p (c w) -> p c w", c=2),
                in_=x_t.ap()[0:2 * P, col0:col0 + P]
                    .rearrange("(c p) w -> p c w", c=2))
            k0, kn = kchunks[2]
            nc.sync.dma_start(out=xt[:kn, 2 * P:3 * P],
                              in_=x_t.ap()[k0:k0 + kn, col0:col0 + P])
            return xt

        def gather(block, mk):
            mk = max(mk, 2)
            g = sbG.tile([P, ROW], HC, tag="g")
            nc.gpsimd.indirect_dma_start(
                out=g[:mk, :], out_offset=None, in_=hc_d.ap(),
                in_offset=bass.IndirectOffsetOnAxis(
                    ap=idx_sb[:mk, block:block + 1], axis=0))
            return g

        def transpose_src(src_ap, dtype, id_ap):
            tp = psT.tile([P, len(mchunks) * P], dtype, tag="tp")
            outs = []
            for ci, (m0, mn) in enumerate(mchunks):
                nc.tensor.transpose(out=tp[:mn, ci * P:ci * P + P],
                                    in_=src_ap[:, m0:m0 + mn], identity=id_ap)
            for ci, (m0, mn) in enumerate(mchunks):
                st = sbC.tile([P, P], HC, tag=f"tT{m0}")
                nc.vector.tensor_copy(out=st[:mn, :], in_=tp[:mn, ci * P:ci * P + P])
                outs.append(st)
            return outs

        def compute_tile(kind, xt, gs, rounds, slot0=None, rrow0=None,
                         scatter_block=None):
            has_rounds = len(rounds) > 0
            has_iouh = (has_rounds or kind == "tail") and kind != "pseudo"

            psum_iou = None
            if kind != "pseudo":
                psum_iou = psA.tile([P, M3], F32, tag="ziou")
                nz = len(kchunks)
                for ci, (k0, kn) in enumerate(kchunks):
                    last = (ci == nz - 1) and not has_iouh
                    nc.tensor.matmul(out=psum_iou[:],
                                     lhsT=xt[:kn, ci * P:ci * P + P],
                                     rhs=w_all_sb[ci][:kn, :M3],
                                     start=(ci == 0), stop=last)
            psum_fx = None
            if has_rounds or kind == "pseudo":
                psum_fx = psB.tile([P, M], F32, tag="zfx")
                for ci, (k0, kn) in enumerate(kchunks):
                    nc.tensor.matmul(out=psum_fx[:],
                                     lhsT=xt[:kn, ci * P:ci * P + P],
                                     rhs=w_all_sb[ci][:kn, M3:M4],
                                     start=(ci == 0), stop=(ci == len(kchunks) - 1))

            acc = None
            if has_rounds or kind in ("pseudo", "tail"):
                acc = sbC.tile([P, 2 * M], F32, tag="acc")
                if kind == "tail":
                    nc.sync.dma_start(out=acc[:],
                                      in_=reduced_d.ap()[rrow0:rrow0 + P, :])
                else:
                    nc.gpsimd.memset(acc[:], 0.0)

            for g, (mk, b) in zip(gs, rounds):
                fp = sbC.tile([P, M], F32, tag="fp")
                nc.vector.tensor_add(out=fp[:mk, :], in0=psum_fx[:mk, :],
                                     in1=g[:mk, M:2 * M])
                f = sbC.tile([P, M], F32, tag="f")
                nc.scalar.activation(out=f[:mk, :], in_=fp[:mk, :], func=SIG)
                fc = sbC.tile([P, M], F32, tag="fc")
                nc.vector.tensor_mul(out=fc[:mk, :], in0=f[:mk, :],
                                     in1=g[:mk, 2 * M:3 * M])
                acc_eng = nc.vector if kind == "tail" else nc.gpsimd
                acc_eng.tensor_add(out=acc[:mk, :M], in0=acc[:mk, :M],
                                   in1=g[:mk, :M])
                acc_eng.tensor_add(out=acc[:mk, M:], in0=acc[:mk, M:],
                                   in1=fc[:mk, :])

            if kind == "pseudo":
                nc.gpsimd.indirect_dma_start(
                    out=partial_d.ap(), in_=acc[:],
                    out_offset=bass.IndirectOffsetOnAxis(
                        ap=idx_sb[:, scatter_block:scatter_block + 1], axis=0),
                    in_offset=None)
                return

            if has_iouh:
                hsT = transpose_src(acc[:, :M], F32, ident_f[:])
                for ci, ((m0, mn), lt) in enumerate(zip(mchunks, hsT)):
                    nc.tensor.matmul(out=psum_iou[:], lhsT=lt[:mn, :],
                                     rhs=w_iouh_sb[ci][:mn, :],
                                     start=False, stop=(ci == len(mchunks) - 1))

            # gates: iou layout [i | o | u]
            gio = sbC.tile([P, 2 * M], F32, tag="gio")
            nc.scalar.activation(out=gio[:], in_=psum_iou[:, :2 * M], func=SIG)
            gu = sbC.tile([P, M], F32, tag="gu")
            nc.scalar.activation(out=gu[:], in_=psum_iou[:, 2 * M:], func=TANH)

            hcrow = sbO.tile([P, ROW], HC, tag="hcrow")
            if acc is not None:
                cg = sbC.tile([P, M], F32, tag="cg")
                nc.vector.tensor_mul(out=cg[:], in0=gio[:, :M], in1=gu[:])
                cg_eng = nc.vector if kind == "tail" else nc.gpsimd
                cg_eng.tensor_add(out=hcrow[:, 2 * M:], in0=cg[:],
                                  in1=acc[:, M:])
            else:
                nc.vector.tensor_mul(out=hcrow[:, 2 * M:], in0=gio[:, :M],
                                     in1=gu[:])
            tc_t = sbC.tile([P, M], F32, tag="tct")
            nc.scalar.activation(out=tc_t[:], in_=hcrow[:, 2 * M:], func=TANH)
            nc.vector.tensor_mul(out=hcrow[:, :M], in0=gio[:, M:2 * M],
                                 in1=tc_t[:])

            hT = transpose_src(hcrow[:, :M], HC, ident[:])
            psum_fp = psC.tile([P, M], F32, tag="fpmm")
            for ci, ((m0, mn), lt) in enumerate(zip(mchunks, hT)):
                nc.tensor.matmul(out=psum_fp[:], lhsT=lt[:mn, :],
                                 rhs=w_fh_sb[ci][:mn, :],
                                 start=(ci == 0), stop=(ci == len(mchunks) - 1))
            nc.any.tensor_copy(out=hcrow[:, M:2 * M], in_=psum_fp[:])

            nc.sync.dma_start(out=hc_d.ap()[slot0:slot0 + P, :], in_=hcrow[:])

        def do_level(kind, tiles_meta, x_col0, base=None, rrow0=None):
            loads = []
            for tm in tiles_meta:
                if kind == "pseudo":
                    (t, scatter_block, rounds) = tm
                else:
                    (t, rounds) = tm
                    scatter_block = None
                xt = xT_load(x_col0 + t * P)
                gs = [gather(b, mk) for (mk, b) in rounds]
                loads.append((t, rounds, scatter_block, xt, gs))
            for (t, rounds, scatter_block, xt, gs) in loads:
                compute_tile(kind, xt, gs, rounds,
                             slot0=None if base is None else base + t * P,
                             rrow0=None if rrow0 is None else rrow0 + t * P,
                             scatter_block=scatter_block)

        # ================= main schedule =================
        for (l, base, nt, tiles_meta) in meta["bulk"]:
            do_level("leaf" if l == 0 else "bulk", tiles_meta, base, base=base)

        if meta["pseudo"]:
            do_level("pseudo", meta["pseudo"], S_slots)

        if s.R_count > 0:
            nc.gpsimd.collective_compute(
                "AllReduce", mybir.AluOpType.add,
                replica_groups=[list(range(n_cores))],
                ins=[partial_d.ap()], outs=[reduced_d.ap()])

        rrow0 = 0
        for (l, base, nt, tiles_meta) in meta["tail"]:
            do_level("tail", tiles_meta, base, base=base, rrow0=rrow0)
            rrow0 += len(s.tail_level_nodes[l])

    nc.compile()
    return nc


def build_inputs(s, idx_streams, nblocks, x, W_ioux, b_ioux, W_iouh,
                 b_iouh, W_fx, b_fx, W_fh, b_fh, mm_dt="f32r", hc_dt="f32r"):
    import ml_dtypes
    np_mm = {"bf16": ml_dtypes.bfloat16, "f32r": np.float32, "f32": np.float32}[mm_dt]
    np_hc = {"bf16": ml_dtypes.bfloat16, "f32r": np.float32, "f32": np.float32}[hc_dt]
    IN = x.shape[1]
    M = W_fh.shape[0]
    x_ts, S_x, pseudo_base = build_x_inputs(s, x, dt=np_mm)

    w_all = np.zeros((IN + 1, 4 * M), np.float32)
    w_all[:IN, :3 * M] = W_ioux.T
    w_all[IN, :3 * M] = b_ioux + b_iouh
    w_all[:IN, 3 * M:] = W_fx.T
    w_all[IN, 3 * M:] = b_fx + b_fh
    w_all = w_all.astype(np_mm)
    w_iouh_t = np.ascontiguousarray(W_iouh.T).astype(np_hc)
    w_fh_t = np.ascontiguousarray(W_fh.T).astype(np_hc)

    in_maps = []
    for c in range(s.n_cores):
        idx_pc = np.ascontiguousarray(
            idx_streams[c][:nblocks * P, 0].reshape(nblocks, P).T)
        in_maps.append({
            "x_t": x_ts[c],
            "idx": idx_pc,
            "w_all_t": w_all,
            "w_iouh_t": w_iouh_t,
            "w_fh_t": w_fh_t,
        })
    return in_maps


def assemble_output(s, results, M=150):
    N = s.N
    h_full = np.zeros((N, M), np.float32)
    for c in range(s.n_cores):
        nodes = np.nonzero(s.core_of[:N] == c)[0]
        if len(nodes):
            h_full[nodes] = results[c]["hc"][s.slot_of[c][nodes], :M]
    tnodes = np.nonzero(s.core_of[:N] < 0)[0]
    if len(tnodes):
        h_full[tnodes] = results[0]["hc"][s.slot_of[0][tnodes], :M]
    return h_full


_PROFILE_STATE = {"exec_ns": None}


def _install_profile_hook():
    import sys, types
    try:
        import antenv.axon_hooks  # noqa: F401
        return True
    except ImportError:
        pass
    try:
        import antenv
        from trn_agent_boot.trn_boot import _ntff_profile_via_ctypes
    except ImportError:
        return False
    hook = _ntff_profile_via_ctypes("/opt/axon/libaxon_pjrt.so")
    if hook is None:
        return False
    mod = types.ModuleType("antenv.axon_hooks")
    state = {"h": hook}
    mod.set_axon_ntff_profile_hook = lambda h: state.__setitem__("h", h)
    mod.get_axon_ntff_profile_hook = lambda: state["h"]
    sys.modules["antenv.axon_hooks"] = mod
    antenv.axon_hooks = mod
    return True


def kernel(x, W_ioux, b_ioux, W_iouh, b_iouh, W_fx, b_fx, W_fh, b_fh, parents):
    import os
    from concourse import bass_utils

    x = np.asarray(x, np.float32)
    parents_np = np.asarray(parents)
    s = build_schedule(parents_np.astype(np.int64), n_cores=8)
    meta, idx_streams, nblocks = build_gather_plan(s)
    nc = build_kernel(s, meta, nblocks, IN=x.shape[1], M=np.asarray(W_fh).shape[0])
    in_maps = build_inputs(s, idx_streams, nblocks, x,
                           np.asarray(W_ioux, np.float32), np.asarray(b_ioux, np.float32),
                           np.asarray(W_iouh, np.float32), np.asarray(b_iouh, np.float32),
                           np.asarray(W_fx, np.float32), np.asarray(b_fx, np.float32),
                           np.asarray(W_fh, np.float32), np.asarray(b_fh, np.float32))
    trace = os.environ.get("TREELSTM_PROFILE", "") == "1"
    if trace:
        trace = _install_profile_hook()
    res = bass_utils.run_bass_kernel_spmd(
        nc, in_maps, core_ids=list(range(8)), trace=trace)
    _PROFILE_STATE["exec_ns"] = res.exec_time_ns
    return assemble_output(s, res.results).astype(np.float32)

